# revision 1
# baseline (speedup 1.0000x reference)
"""Trainium2 Bass kernel for nn_CrossAttention (B=8, S=2048, D=512, fp32).

Sharding: data-parallel over batch across the 8 NeuronCores (one batch
element per core); the 512x512 projection weights are replicated.

Per-core dataflow (matmul inputs in bf16, fp32 PSUM accumulation):
  1. PE-transpose z/W fp32 blocks (cast to bf16 on the PSUM->SBUF copy) so
     the contraction dim (d) lands on partitions.
  2. Projections:  Qt[e,sq] = WqT.T @ zqT (+bq), Kt[e,sk] likewise,
                   V[sk,e]  = zvT.T @ WvT (+bv)
  3. Per 512-wide sq macro-tile:
       scoresT[sk, sq] = Kt.T @ Qt  (PSUM fp32) -> exp(./sqrt(D)) -> bf16
       per 128-row sq subtile:
         AV   psum[sq,e]  = sum_skc expT_chunk.T @ V_chunk
         rsum psum[sq,2]  = sum_skc expT_chunk.T @ ones
         ot = AV * (1/rsum); bn_stats/aggr -> stash mean/var
  4. Single batched Sqrt+reciprocal for all row stats (one ACT table load),
     then per subtile: (ot-mu)*rstd*gamma+beta -> DMA out.
  Softmax skips the max-subtraction: scores ~ N(0,1), so exp() is safely
  within fp32 range; matches jax softmax up to rounding.
"""

import math
import os
import sys
from contextlib import ExitStack

for _p in ("/opt/trn_rl_repo", "/root/.axon_site/_ro/trn_rl_repo"):
    if os.path.isdir(_p) and _p not in sys.path:
        sys.path.append(_p)

import numpy as np

import concourse.bacc as bacc
import concourse.bass as bass
import concourse.mybir as mybir
import concourse.tile as tile
from concourse.bass import ds, ts
from concourse.bass_utils import run_bass_kernel_spmd
from concourse.masks import make_identity

P = 128
B = 8
S = 2048
D = 512
DC = D // P       # 4   chunks of the model dim
SC = S // P       # 16  chunks of the sequence dim
NQ = 512          # sq macro-tile width (matmul free dim)
NT = S // NQ      # 4   macro tiles
LN_EPS = 1e-5
F32 = mybir.dt.float32
BF16 = mybir.dt.bfloat16

INPUT_NAMES = (
    "z_q", "z_k", "z_v", "Wq", "bq", "Wk", "bk", "Wv", "bv",
    "ln_gamma", "ln_beta",
)


def _bcast_row_load(nc, dst, src_1d):
    """DMA-replicate a [D] DRAM vector across all partitions of dst [P, D]."""
    src = bass.AP(
        tensor=src_1d.tensor,
        offset=src_1d.offset,
        ap=[[0, dst.shape[0]]] + list(src_1d.ap),
    )
    nc.gpsimd.dma_start(out=dst, in_=src)


def _build_tile_kernel(tc, ins, out):
    nc = tc.nc
    z_q, z_k, z_v, Wq, bq, Wk, bk, Wv, bv, ln_g, ln_b = (ins[k] for k in INPUT_NAMES)

    ctx = ExitStack()
    singles = ctx.enter_context(tc.tile_pool(name="singles", bufs=1))

    ident = singles.tile([P, P], F32)
    make_identity(nc, ident)
    ident16 = singles.tile([P, P], BF16)
    nc.vector.tensor_copy(ident16, ident)

    Qt = singles.tile([P, DC, S], BF16)   # [e_in, e_out, sq]
    Kt = singles.tile([P, DC, S], BF16)   # [e_in, e_out, sk]
    V = singles.tile([P, SC, D], BF16)    # [sk_in, sk_out, e]

    # ------------- phase 1: PE transposes (cast to bf16) + projections ------
    inv_sqrt_d = 1.0 / math.sqrt(D)
    outr = out.rearrange("(so p) d -> p so d", p=P)
    zt_q = singles.tile([P, DC, S], BF16)  # q stays live through phase 2
    with (
        tc.tile_pool(name="wz", bufs=3) as wz,
        tc.tile_pool(name="wp", bufs=2) as wp,
        tc.tile_pool(name="ztp", bufs=2) as ztp,
        tc.tile_pool(name="ps_tp", bufs=5, space="PSUM") as ps_tp,
        tc.tile_pool(name="ps13", bufs=3, space="PSUM") as ps13,
    ):
        def transpose_w(W, name):
            # wt[d_in, d_out, e] bf16, via PE transpose of bf16 blocks
            w_nat = wp.tile([P, DC, D], F32, tag="wnat", name="w_nat")
            nc.sync.dma_start(w_nat, W.rearrange("(eo p) d -> p eo d", p=P))
            w_n16 = wp.tile([P, DC, D], BF16, tag="wnat16", name="w_n16")
            nc.vector.tensor_copy(w_n16, w_nat)
            wt = singles.tile([P, DC, D], BF16, tag=f"wt_{name}", name=f"wt_{name}")
            for do in range(DC):
                pt = ps_tp.tile([P, DC, P], BF16, tag="tp", name="pt")
                for eo in range(DC):
                    nc.tensor.transpose(
                        pt[:, eo, :], w_n16[:, eo, ts(do, P)], ident16
                    )
                nc.vector.tensor_copy(wt[:, do, :], pt)
            return wt

        def transpose_z(z, zt, engines=(None, None)):
            # zt[d_in, d_out, s] bf16; ladder-sized chunks (small first so the
            # PE starts early), loads+casts emitted ahead of the transposes.
            sizes = (1, 1, 2, 4, 4, 4)   # 128-row groups per chunk, sum = 16
            zr = z.rearrange("(g p) d -> p g d", p=P)
            zn16s = []
            g0 = 0
            for c, jc in enumerate(sizes):
                znat = wz.tile([P, 4, D], F32, tag="znat", name="znat")[:, :jc]
                eng = engines[c % 2] or (nc.scalar if c % 2 == 0 else nc.sync)
                eng.dma_start(znat, zr[:, ds(g0, jc), :])
                zn16 = wz.tile([P, 4, D], BF16, tag="zn16", name="zn16")[:, :jc]
                nc.vector.tensor_copy(zn16, znat)
                zn16s.append((g0, jc, zn16))
                g0 += jc
            for g0, jc, zn16 in zn16s:
                for do in range(DC):
                    pt = ps_tp.tile([P, 4, P], BF16, tag="tp", name="pt")[:, :jc]
                    for j in range(jc):
                        nc.tensor.transpose(
                            pt[:, j, :], zn16[:, j, ts(do, P)], ident16
                        )
                    nc.vector.tensor_copy(zt[:, do, ds(g0 * P, jc * P)], pt)
            return zt

        def project_qk(zt, wt, bias_sb, dst, sn_range):
            # dst[e, s] = W @ z.T + b   laid out [P, DC(e_out), S]
            for sn in sn_range:
                for eo in range(DC):
                    ps = ps13.tile([P, NQ], F32, tag="proj")
                    for do in range(DC):
                        nc.tensor.matmul(
                            ps,
                            wt[:, do, ts(eo, P)],
                            zt[:, do, ts(sn, NQ)],
                            start=(do == 0),
                            stop=(do == DC - 1),
                        )
                    nc.vector.tensor_scalar_add(
                        dst[:, eo, ts(sn, NQ)], ps, bias_sb[:, eo : eo + 1]
                    )

        # K first (scores need all of Kt), then V, then Q fused into phase 2.
        zt_k = ztp.tile([P, DC, S], BF16, tag="zt", name="zt_k")
        transpose_z(z_k, zt_k)
        wt_k = transpose_w(Wk, "k")

        # small constants / biases: gpsimd SWDGE, off the hot HWDGE queues
        ones_f32 = singles.tile([P, 2], F32)
        nc.vector.memset(ones_f32, 1.0)
        ones = singles.tile([P, 2], BF16)
        nc.vector.tensor_copy(ones, ones_f32)
        eps_sb = singles.tile([P, 1], F32)
        nc.vector.memset(eps_sb, LN_EPS)
        bq_sb = singles.tile([P, DC], F32)
        nc.gpsimd.dma_start(bq_sb, bq.rearrange("(eo p) -> p eo", p=P))
        bk_sb = singles.tile([P, DC], F32)
        nc.gpsimd.dma_start(bk_sb, bk.rearrange("(eo p) -> p eo", p=P))
        bv_sb = singles.tile([P, D], F32)
        _bcast_row_load(nc, bv_sb, bv)
        gam_sb = singles.tile([P, D], F32)
        _bcast_row_load(nc, gam_sb, ln_g)
        bet_sb = singles.tile([P, D], F32)
        _bcast_row_load(nc, bet_sb, ln_b)

        project_qk(zt_k, wt_k, bk_sb, Kt, range(S // NQ))

        zt_v = ztp.tile([P, DC, S], BF16, tag="zt", name="zt_v")
        transpose_z(z_v, zt_v)
        wt_v = transpose_w(Wv, "v")
        # V[sk, e] = z_v @ Wv.T + bv   laid out [P, SC(sk_out), D]
        for sko in range(SC):
            ps = ps13.tile([P, D], F32, tag="proj")
            for do in range(DC):
                nc.tensor.matmul(
                    ps,
                    zt_v[:, do, ts(sko, P)],
                    wt_v[:, do, :],
                    start=(do == 0),
                    stop=(do == DC - 1),
                )
            nc.vector.tensor_tensor(V[:, sko, :], ps, bv_sb, mybir.AluOpType.add)

        transpose_z(z_q, zt_q)
        wt_q = transpose_w(Wq, "q")

    # ---------------- phase 2: Q-projection + attention + layernorm ---------
    with (
        tc.tile_pool(name="expp", bufs=3) as expp,
        tc.tile_pool(name="otp", bufs=8) as otp,
        tc.tile_pool(name="ep", bufs=4) as ep,
        tc.tile_pool(name="ps_sc", bufs=3, space="PSUM") as ps_sc,
        tc.tile_pool(name="ps_av", bufs=3, space="PSUM") as ps_av,
        tc.tile_pool(name="ps_rs", bufs=2, space="PSUM") as ps_rs,
    ):
        stats = singles.tile([P, SC, 2], F32)   # per-subtile (mean, var)
        rstd_all = singles.tile([P, SC], F32)
        def project_q_slice(tq):
            for eo in range(DC):
                psq = ps_sc.tile([P, NQ], F32, tag="sc", name="psq")
                for do in range(DC):
                    nc.tensor.matmul(
                        psq,
                        wt_q[:, do, ts(eo, P)],
                        zt_q[:, do, ts(tq, NQ)],
                        start=(do == 0),
                        stop=(do == DC - 1),
                    )
                nc.vector.tensor_scalar_add(
                    Qt[:, eo, ts(tq, NQ)], psq, bq_sb[:, eo : eo + 1]
                )

        ots = []
        project_q_slice(0)
        for tq in range(NT):
            expT = expp.tile([P, SC, NQ], BF16, tag="expT")  # [sk_in,sk_out,sq]
            for skc in range(SC):
                pss = ps_sc.tile([P, NQ], F32, tag="sc")
                for eo in range(DC):
                    nc.tensor.matmul(
                        pss,
                        Kt[:, eo, ts(skc, P)],
                        Qt[:, eo, ts(tq, NQ)],
                        start=(eo == 0),
                        stop=(eo == DC - 1),
                    )
                nc.scalar.activation(
                    expT[:, skc, :], pss,
                    mybir.ActivationFunctionType.Exp,
                    scale=inv_sqrt_d,
                )
            if tq + 1 < NT:
                project_q_slice(tq + 1)
            for m in range(NQ // P):
                so = tq * (NQ // P) + m  # global 128-row subtile index
                pso = ps_av.tile([P, D], F32, tag="av")
                psr = ps_rs.tile([P, 2], F32, tag="rs")
                for skc in range(SC):
                    lhsT = expT[:, skc, ts(m, P)]
                    nc.tensor.matmul(
                        pso, lhsT, V[:, skc, :],
                        start=(skc == 0), stop=(skc == SC - 1),
                    )
                    nc.tensor.matmul(
                        psr, lhsT, ones,
                        start=(skc == 0), stop=(skc == SC - 1),
                    )
                rinv = ep.tile([P, 1], F32, tag="rinv")
                nc.vector.reciprocal(rinv, psr[:, 0:1])
                ot = otp.tile([P, D], F32, tag="ot")
                nc.vector.tensor_scalar_mul(ot, pso, rinv)
                st6 = ep.tile([P, 6], F32, tag="st6")
                nc.vector.bn_stats(st6, ot)
                nc.vector.bn_aggr(stats[:, so, :], st6)
                ots.append(ot)
            # per-macro-tile epilogue: batch Sqrt over this tile's 4 subtiles
            mslice = ds(tq * (NQ // P), NQ // P)
            nc.scalar.activation(
                rstd_all[:, mslice], stats[:, mslice, 1],
                mybir.ActivationFunctionType.Sqrt,
                bias=eps_sb,
            )
            nc.vector.reciprocal(rstd_all[:, mslice], rstd_all[:, mslice])
            for m in range(NQ // P):
                so = tq * (NQ // P) + m
                ot = ots[so]
                nc.vector.tensor_scalar(
                    ot, ot, stats[:, so, 0:1], rstd_all[:, so : so + 1],
                    op0=mybir.AluOpType.subtract,
                    op1=mybir.AluOpType.mult,
                )
                nc.vector.tensor_tensor(ot, ot, gam_sb, mybir.AluOpType.mult)
                nc.vector.tensor_tensor(ot, ot, bet_sb, mybir.AluOpType.add)
                nc.sync.dma_start(outr[:, so, :], ot)
    ctx.close()


_NC_CACHE = None


def _build():
    global _NC_CACHE
    if _NC_CACHE is not None:
        return _NC_CACHE
    nc = bacc.Bacc("TRN2", target_bir_lowering=False, debug=False, num_devices=B)
    shapes = {
        "z_q": [S, D], "z_k": [S, D], "z_v": [S, D],
        "Wq": [D, D], "Wk": [D, D], "Wv": [D, D],
        "bq": [D], "bk": [D], "bv": [D],
        "ln_gamma": [D], "ln_beta": [D],
    }
    ins = {
        k: nc.dram_tensor(k, shapes[k], F32, kind="ExternalInput").ap()
        for k in INPUT_NAMES
    }
    out = nc.dram_tensor("out", [S, D], F32, kind="ExternalOutput").ap()
    with tile.TileContext(nc) as tc:
        _build_tile_kernel(tc, ins, out)
    nc.compile()
    _NC_CACHE = nc
    return nc


def _run(inputs, **spmd_kwargs):
    nc = _build()
    arrs = {k: np.ascontiguousarray(np.asarray(inputs[k]), dtype=np.float32)
            for k in INPUT_NAMES}
    in_maps = []
    for b in range(B):
        m = {"z_q": arrs["z_q"][b], "z_k": arrs["z_k"][b], "z_v": arrs["z_v"][b]}
        for k in ("Wq", "bq", "Wk", "bk", "Wv", "bv", "ln_gamma", "ln_beta"):
            m[k] = arrs[k]
        in_maps.append(m)
    res = run_bass_kernel_spmd(nc, in_maps, core_ids=list(range(B)), **spmd_kwargs)
    out = np.stack([res.results[b]["out"] for b in range(B)], axis=0)
    return out, res


def kernel(**inputs):
    out, _ = _run(inputs)
    return out

